# revision 1
# baseline (speedup 1.0000x reference)
"""FAVOR+ attention (Performer) Trainium2 kernel, 8-way sharded.

Sharding: 8 cores = 4 batches x 2 head-groups. Core c handles batch c//2 and
heads [8*(c%2), 8*(c%2)+8). The attention core (kv state) is fully local per
head; the output projection is computed as a per-core partial over its 512
input channels and the two partials per batch are summed on the host.

The q/k feature GEMMs run in fp8e4 with DoubleRow (2 rows/cell, 2x PE
throughput); their rounding error enters the exp() argument scaled by ~0.2 so
the output error stays at the few-1e-3 level. The v and proj GEMMs stay bf16
(their error hits the output linearly). Accumulation is fp32 in PSUM.
"""

import numpy as np
import ml_dtypes

import concourse.mybir as mybir
import concourse.tile as tile
from concourse import bacc
from concourse.bass_utils import run_bass_kernel_spmd

F32 = mybir.dt.float32
F16 = mybir.dt.float16
BF16 = mybir.dt.bfloat16
FP8 = mybir.dt.float8e4
AF = mybir.ActivationFunctionType
ALU = mybir.AluOpType
DR = mybir.MatmulPerfMode.DoubleRow

N = 4096
D = 1024
HD = 64
NF = 64
EPS = 1e-4
BLK = 512  # n-block
NBLK = N // BLK
NCH = BLK // 128  # 128-row chunks per block
SCALE = float(HD) ** -0.25
WS = 128.0  # fp8 weight pre-scale (undone in the exp activation's scale)


def _build_nc():
    nc = bacc.Bacc("TRN2", target_bir_lowering=False, debug=False, num_devices=8)

    xt = nc.dram_tensor("xt", [NBLK, 128, 8, BLK], BF16, kind="ExternalInput").ap()
    xt8 = nc.dram_tensor("xt8", [NBLK, 128, 8, BLK], FP8, kind="ExternalInput").ap()
    wqp8 = nc.dram_tensor("wqp8", [128, 8, 512], FP8, kind="ExternalInput").ap()
    wkp8 = nc.dram_tensor("wkp8", [128, 8, 512], FP8, kind="ExternalInput").ap()
    wv = nc.dram_tensor("wv", [128, 8, 512], BF16, kind="ExternalInput").ap()
    wp = nc.dram_tensor("wp", [128, 4, 1024], BF16, kind="ExternalInput").ap()
    bqpe = nc.dram_tensor("bqpe", [128, 4], F32, kind="ExternalInput").ap()
    bkpb = nc.dram_tensor("bkpb", [128, 8, 64], BF16, kind="ExternalInput").ap()
    bvb = nc.dram_tensor("bvb", [128, 4, 64], F32, kind="ExternalInput").ap()
    oh2 = nc.dram_tensor("oh2", [2, 128], BF16, kind="ExternalInput").ap()
    out = nc.dram_tensor("out", [D, N], F16, kind="ExternalOutput").ap()

    out_v = out.rearrange("(oc p) n -> p oc n", p=128)  # [128, 8, 4096]

    with tile.TileContext(nc) as tc:
        with (
            tc.tile_pool(name="consts", bufs=1) as consts,
            tc.tile_pool(name="xp", bufs=3) as xp,
            tc.tile_pool(name="xp8", bufs=4) as xp8,
            tc.tile_pool(name="work", bufs=2) as work,
            tc.tile_pool(name="small", bufs=4) as small,
            tc.tile_pool(name="pbig", bufs=6, space="PSUM") as pbig,
        ):
            pkv = tc.alloc_tile_pool(name="pkv", bufs=1, space="PSUM")
            # ---- pass-A-critical loads first: fp8 k-weights, then bias/v ----
            wkp8_sb = consts.tile([128, 8, 512], FP8, name="wkp8_sb")
            nc.scalar.dma_start(wkp8_sb[:], wkp8)
            bkpb_sb = consts.tile([128, 8, 64], BF16, name="bkpb_sb")
            nc.scalar.dma_start(bkpb_sb[:], bkpb)
            wv_sb = consts.tile([128, 8, 512], BF16, name="wv_sb")
            nc.scalar.dma_start(wv_sb[:, 0:4, :], wv[:, 0:4, :])
            nc.scalar.dma_start(wv_sb[:, 4:8, :], wv[:, 4:8, :])
            eps_sb = consts.tile([128, 1], F32, name="eps_sb")
            nc.vector.memset(eps_sb[:], EPS)
            wtmp = consts.tile([128, 512], BF16, name="wtmp")
            nc.vector.memset(wtmp[:], 0.125)

            # declared now, loaded during pass A
            wqp8_sb = consts.tile([128, 8, 512], FP8, name="wqp8_sb")
            wp_sb = consts.tile([128, 4, 1024], BF16, name="wp_sb")
            oh2_sb = consts.tile([2, 128], BF16, name="oh2_sb")
            bqpe_sb = consts.tile([128, 4], F32, name="bqpe_sb")
            bvb_sb = consts.tile([128, 4, 64], F32, name="bvb_sb")

            # kv accumulators: pairs (0,1) in kvacc0, (2,3) in kvacc1.
            # Layout per pair: 129 cols (64 v-head0 | 64 v-head1 | ksum), stride 130.
            kvacc = [
                pkv.tile([128, 260], F32, name=f"kvacc{t}", tag=f"kvacc{t}")
                for t in range(2)
            ]

            # PE warmup: keep the HAM activity window busy while the first
            # DMAs stream, so real matmuls start at 2.4 GHz.
            ps_warm = pbig.tile([128, 512], F32, name="ps_warm", tag="big")
            for _ in range(18):
                nc.tensor.matmul(
                    ps_warm[:], wtmp[:, 0:128], wtmp[:], start=True, stop=True
                )

            # ================= pass A: k', v -> kv, ksum =================
            def load_xt8(blk):
                t = xp8.tile([128, 8, BLK], FP8, name="xt8_t", tag="xt8")
                nc.sync.dma_start(t[:], xt8[blk])
                return t

            def load_xt(blk):
                t = xp.tile([128, 8, BLK], BF16, name="xt_t", tag="xt")
                nc.sync.dma_start(t[:], xt[blk])
                return t

            def emit_v(xt_t, c, v_sbs):
                cs = slice(c * 128, (c + 1) * 128)
                psv = pbig.tile([128, 512], F32, name="ps_v", tag="big")
                for dc in range(8):
                    nc.tensor.matmul(
                        psv[:],
                        xt_t[:, dc, cs],
                        wv_sb[:, dc, :],
                        start=(dc == 0),
                        stop=(dc == 7),
                    )
                v_sb = work.tile([128, 4, 132], BF16, name="v_sb", tag="v", bufs=5)
                nc.scalar.copy(
                    v_sb[:, :, 0:128],
                    psv.rearrange("p (g j) -> p g j", j=128),
                )
                nc.vector.memset(v_sb[:, :, 128:129], 1.0)
                v_sbs.append(v_sb)

            def emit_kf(x8_t, c, kp_sbs):
                cs = slice(c * 128, (c + 1) * 128)
                psf = pbig.tile([128, 512], F32, name="ps_kf", tag="big")
                for k in range(4):
                    nc.tensor.matmul(
                        psf[:],
                        x8_t[:, 2 * k : 2 * k + 2, cs],
                        wkp8_sb[:, 2 * k : 2 * k + 2, :],
                        start=(k == 0),
                        stop=(k == 3),
                        perf_mode=DR,
                    )
                psf_v = psf.rearrange("p (g f) -> p g f", f=64)  # [128, 8, 64]
                karg = small.tile([128, 8, 64], F16, name="karg", tag="karg")
                nc.vector.tensor_tensor(karg[:], psf_v, bkpb_sb[:], ALU.add)
                mx = small.tile([128, 8], F32, name="mx", tag="mx")
                nc.vector.reduce_max(mx[:], karg[:], axis=mybir.AxisListType.X)
                nc.vector.tensor_tensor(
                    karg[:], karg[:],
                    mx[:, :, None].to_broadcast([128, 8, 64]),
                    ALU.subtract,
                )
                kp_sb = work.tile([128, 4, 128], BF16, name="kp_sb", tag="kp", bufs=9)
                nc.scalar.activation(
                    kp_sb.rearrange("p g (h f) -> p (g h) f", f=64),
                    karg[:], AF.Exp, bias=eps_sb[:], scale=1.0 / WS,
                )
                kp_sbs.append(kp_sb)

            def emit_kv(blk, c, kp_sbs, v_sbs):
                glob_first = blk == 0 and c == 0
                glob_last = blk == NBLK - 1 and c == NCH - 1
                for p in range(4):
                    base = (p % 2) * 130
                    nc.tensor.matmul(
                        kvacc[p // 2][:, base : base + 129],
                        kp_sbs[c][:, p, :],
                        v_sbs[c][:, p, 0:129],
                        start=(glob_first and p % 2 == 0),
                        stop=(glob_last and p % 2 == 1),
                    )

            # blocks 0+1: k-features first (only needs the small fp8 DMAs),
            # so the PE has work while wv / bf16-x are still streaming in.
            x8_0 = load_xt8(0)
            x8_1 = load_xt8(1)
            xt_0 = load_xt(0)
            xt_1 = load_xt(1)
            kp0, v0, kp1, v1 = [], [], [], []
            for c in range(NCH):
                emit_kf(x8_0, c, kp0)
            for c in range(NCH):
                emit_kf(x8_1, c, kp1)
            for c in range(NCH):
                emit_v(xt_0, c, v0)
            for c in range(NCH):
                emit_kv(0, c, kp0, v0)
            for c in range(NCH):
                emit_v(xt_1, c, v1)
            for c in range(NCH):
                emit_kv(1, c, kp1, v1)

            for blk in range(2, NBLK):
                x8_t = load_xt8(blk)
                xt_t = load_xt(blk)
                if blk == 4:
                    # stream pass-B weights once startup DMA pressure is
                    # over (gpsimd SWDGE; sync keeps feeding xt blocks)
                    nc.gpsimd.dma_start(wqp8_sb[:], wqp8)
                    nc.gpsimd.dma_start(wp_sb[:], wp)
                    nc.gpsimd.dma_start(oh2_sb[:], oh2)
                    nc.gpsimd.dma_start(bqpe_sb[:], bqpe)
                    nc.gpsimd.dma_start(bvb_sb[:], bvb)
                v_sbs, kp_sbs = [], []
                for c in range(NCH):
                    emit_kf(x8_t, c, kp_sbs)
                    emit_v(xt_t, c, v_sbs)
                for c in range(NCH):
                    emit_kv(blk, c, kp_sbs, v_sbs)

            # ============ assemble kv blockdiag + ksum columns ============
            kvbd = consts.tile([128, 4, 128], BF16, name="kvbd")
            ksbc = consts.tile([128, 4, 2], BF16, name="ksbc")
            nc.vector.memset(kvbd[:], 0.0)
            nc.vector.memset(ksbc[:], 0.0)
            for p in range(4):
                t = kvacc[p // 2]
                base = (p % 2) * 130
                ks = t[:, base + 128 : base + 129]
                nc.vector.tensor_copy(out=ksbc[0:64, p, 0:1], in_=ks[0:64])
                nc.vector.tensor_copy(out=ksbc[64:128, p, 1:2], in_=ks[64:128])
                # kv[h] += ksum[h] (x) bv[h], fold v-bias into kv
                nc.vector.scalar_tensor_tensor(
                    out=kvbd[0:64, p, 0:64],
                    in0=bvb_sb[0:64, p, :],
                    scalar=ks[0:64],
                    in1=t[0:64, base : base + 64],
                    op0=ALU.mult,
                    op1=ALU.add,
                )
                nc.vector.scalar_tensor_tensor(
                    out=kvbd[64:128, p, 64:128],
                    in0=bvb_sb[64:128, p, :],
                    scalar=ks[64:128],
                    in1=t[64:128, base + 64 : base + 128],
                    op0=ALU.mult,
                    op1=ALU.add,
                )

            # kv accumulator banks are dead now; reuse them for the
            # normalizer tiles of pass B.
            pkv.release()
            pnrm = tc.alloc_tile_pool(name="pnrm", bufs=2, space="PSUM")

            # ================= pass B: q', out, proj =================
            # q_proj^T comes straight from x @ Wqp (feature projection fused
            # into the weights on the host, fp8 DoubleRow); exp bias carries
            # bqp + eps and the 1/WS weight descale rides the exp scale.
            # The nrm -> bc -> po chain has an ACT/DVE hop between stages;
            # interleave each stage with one q-projection group of the NEXT
            # block so the PE never idles (keeps HAM at 2.4 GHz).
            def emit_qp_start(blk):
                x8_t = xp8.tile([128, 8, BLK], FP8, name="xt8_t2", tag="xt8")
                nc.sync.dma_start(x8_t[:], xt8[blk])
                qp_sb = work.tile([128, 4, BLK], BF16, name="qp_sb", tag="qp", bufs=4)
                return x8_t, qp_sb

            def emit_qp_group(x8_t, qp_sb, p):
                ps = pbig.tile([128, BLK], F32, name="ps_qt", tag="big")
                for k in range(4):
                    nc.tensor.matmul(
                        ps[:],
                        wqp8_sb[:, 2 * k : 2 * k + 2, p * 128 : (p + 1) * 128],
                        x8_t[:, 2 * k : 2 * k + 2, :],
                        start=(k == 0),
                        stop=(k == 3),
                        perf_mode=DR,
                    )
                nc.scalar.activation(
                    qp_sb[:, p, :], ps[:], AF.Exp,
                    bias=bqpe_sb[:, p : p + 1], scale=1.0 / WS,
                )

            def emit_pj(blk, o_sb, oc_range):
                ns = slice(blk * BLK, (blk + 1) * BLK)
                for oc in oc_range:
                    pj = pbig.tile([128, BLK], F32, name="ps_pj", tag="big")
                    for jc in range(4):
                        nc.tensor.matmul(
                            pj[:],
                            wp_sb[:, jc, oc * 128 : (oc + 1) * 128],
                            o_sb[:, jc, :],
                            start=(jc == 0),
                            stop=(jc == 3),
                        )
                    pj_sb = small.tile([128, BLK], F16, name="pj_sb", tag="pj", bufs=6)
                    if oc % 2 == 0:
                        nc.vector.tensor_copy(out=pj_sb[:], in_=pj[:])
                        nc.sync.dma_start(out_v[:, oc, ns], pj_sb[:])
                    else:
                        nc.scalar.copy(pj_sb[:], pj[:])
                        nc.scalar.dma_start(out_v[:, oc, ns], pj_sb[:])

            # two-ahead q-projection pipeline: blocks b and b+1 are
            # queued before block b's nrm stage, so the PE has ~7us of
            # queued matmuls to cover the DVE kvbd/ksbc assembly at the
            # pass A -> B transition (keeps HAM at 2.4 GHz through it).
            pipe = [emit_qp_start(0), emit_qp_start(1)]
            for p in range(4):
                emit_qp_group(pipe[0][0], pipe[0][1], p)
            for p in range(4):
                emit_qp_group(pipe[1][0], pipe[1][1], p)

            prev_o = None  # (blk, o_sb) whose proj is still pending
            for blk in range(NBLK):
                qp_sb = pipe[blk][1]
                if blk + 2 < NBLK:
                    fut = emit_qp_start(blk + 2)
                    pipe.append(fut)

                    def filler(stage, fut=fut):
                        emit_qp_group(fut[0], fut[1], stage)

                    flush = True
                else:
                    # last two blocks: no q-projection left; fill with the
                    # pending proj of the previous block
                    lo_blk, lo_sb = prev_o
                    prev_o = None

                    def filler(stage, lo_blk=lo_blk, lo_sb=lo_sb):
                        emit_pj(lo_blk, lo_sb, range(stage * 2, stage * 2 + 2))

                    flush = False

                # normalizer -> 1/norm (approx, 18 bits) -> bf16
                rns = []
                for p in range(4):
                    nrm = pnrm.tile([2, BLK], F32, name="nrm", tag="nrm")
                    nc.tensor.matmul(
                        nrm[:], ksbc[:, p, :], qp_sb[:, p, :],
                        start=True, stop=True,
                    )
                    rf = small.tile([2, BLK], F32, name="rf", tag="rf")
                    nc.vector.reciprocal_approx_fast(out=rf[:], in_=nrm[:])
                    rn = small.tile([2, BLK], BF16, name="rn", tag="rn")
                    if p % 2 == 0:
                        nc.scalar.copy(rn[:], rf[:])
                    else:
                        nc.vector.tensor_copy(out=rn[:], in_=rf[:])
                    rns.append(rn)
                filler(0)

                # broadcast 1/norm over each head's 64 partitions; divide q'
                q2s = []
                for p in range(4):
                    bc = pbig.tile([128, BLK], F32, name="ps_bc", tag="big")
                    nc.tensor.matmul(
                        bc[:], oh2_sb[:], rns[p][:], start=True, stop=True
                    )
                    q2 = small.tile([128, BLK], BF16, name="q2", tag="q2", bufs=6)
                    nc.vector.tensor_mul(q2[:], qp_sb[:, p, :], bc[:])
                    q2s.append(q2)
                filler(1)

                o_sb = work.tile([128, 4, BLK], BF16, name="o_sb", tag="o", bufs=3)
                for p in range(4):
                    po = pbig.tile([128, BLK], F32, name="ps_o", tag="big")
                    nc.tensor.matmul(
                        po[:], kvbd[:, p, :], q2s[p][:], start=True, stop=True
                    )
                    if p % 2 == 0:
                        nc.scalar.copy(o_sb[:, p, :], po[:])
                    else:
                        nc.vector.tensor_copy(out=o_sb[:, p, :], in_=po[:])
                filler(2)
                filler(3)

                # flush the previous block's pending proj, keep ours pending
                if flush and prev_o is not None:
                    emit_pj(prev_o[0], prev_o[1], range(8))
                prev_o = (blk, o_sb)

            # proj of the final block
            emit_pj(prev_o[0], prev_o[1], range(8))

            pnrm.release()

    nc.compile()
    return nc


_NC = None


def _get_nc():
    global _NC
    if _NC is None:
        _NC = _build_nc()
    return _NC


def _host_inputs(x, W_qkv, b_qkv, W_proj, b_proj, proj_mat):
    x = np.asarray(x, dtype=np.float32)
    W_qkv = np.asarray(W_qkv, dtype=np.float32)
    b_qkv = np.asarray(b_qkv, dtype=np.float32)
    W_proj = np.asarray(W_proj, dtype=np.float32)
    proj_mat = np.asarray(proj_mat, dtype=np.float32)

    pt = (proj_mat.T * SCALE).astype(np.float32)  # [hd, F]
    oh2 = np.zeros((2, 128), dtype=np.float32)
    oh2[0, :64] = 1.0
    oh2[1, 64:] = 1.0

    def tile_x(xb):
        # [NBLK, 128, 8, BLK]: contiguous per-partition runs for fast DMA
        xt = xb.T.reshape(8, 128, NBLK, BLK).transpose(2, 1, 0, 3)
        return np.ascontiguousarray(xt)

    def tile_w(w):
        # [D, 512] -> [128, 8, 512]
        return np.ascontiguousarray(w.reshape(8, 128, 512).transpose(1, 0, 2))

    xts_f32 = [tile_x(x[b]) for b in range(4)]
    xts = [t.astype(ml_dtypes.bfloat16) for t in xts_f32]
    xt8s = [t.astype(ml_dtypes.float8_e4m3) for t in xts_f32]

    def fuse(Wslc, bslc):
        # W_fused[:, (h f)] = sum_d W.T[:, (h d)] pt[d, f]; bias likewise
        wT = Wslc.T.reshape(D, 8, HD)
        wf = np.einsum("ahd,df->ahf", wT, pt).reshape(D, 512)
        bf = np.einsum("hd,df->hf", bslc.reshape(8, HD), pt).reshape(512)
        return wf, bf

    in_maps = []
    for c in range(8):
        b, g = c // 2, c % 2
        wqs = W_qkv[g * 512 : (g + 1) * 512]
        wks = W_qkv[D + g * 512 : D + (g + 1) * 512]
        wvs = W_qkv[2 * D + g * 512 : 2 * D + (g + 1) * 512]
        bqs = b_qkv[g * 512 : (g + 1) * 512]
        bks = b_qkv[D + g * 512 : D + (g + 1) * 512]
        bvs = b_qkv[2 * D + g * 512 : 2 * D + (g + 1) * 512]
        wqp, bqp = fuse(wqs, bqs)
        wkp, bkp = fuse(wks, bks)
        bvb = np.empty((128, 4, 64), dtype=np.float32)
        bv_r = bvs.reshape(4, 2, 64)
        for p in range(4):
            bvb[0:64, p, :] = bv_r[p, 0][None, :]
            bvb[64:128, p, :] = bv_r[p, 1][None, :]
        in_maps.append(
            {
                "xt": xts[b],
                "xt8": xt8s[b],
                "wqp8": tile_w(wqp * WS).astype(ml_dtypes.float8_e4m3),
                "wkp8": tile_w(wkp * WS).astype(ml_dtypes.float8_e4m3),
                "wv": tile_w(np.ascontiguousarray(wvs.T)).astype(ml_dtypes.bfloat16),
                "wp": np.ascontiguousarray(
                    W_proj[:, g * 512 : (g + 1) * 512].T.reshape(4, 128, 1024)
                    .transpose(1, 0, 2)
                ).astype(ml_dtypes.bfloat16),
                "bqpe": np.ascontiguousarray(
                    (bqp + EPS).reshape(4, 128).T
                ).astype(np.float32),
                "bkpb": np.ascontiguousarray(
                    np.broadcast_to(bkp.reshape(1, 8, 64) * WS, (128, 8, 64))
                ).astype(ml_dtypes.bfloat16),
                "bvb": bvb,
                "oh2": oh2.astype(ml_dtypes.bfloat16),
            }
        )
    return in_maps


def kernel(x, W_qkv, b_qkv, W_proj, b_proj, proj_mat):
    b_proj = np.asarray(b_proj, dtype=np.float32)
    in_maps = _host_inputs(x, W_qkv, b_qkv, W_proj, b_proj, proj_mat)
    nc = _get_nc()
    res = run_bass_kernel_spmd(nc, in_maps, core_ids=list(range(8)))
    final = np.empty((4, N, D), dtype=np.float32)
    for b in range(4):
        acc = res.results[2 * b]["out"].astype(np.float32) + res.results[
            2 * b + 1
        ]["out"].astype(np.float32)
        final[b] = acc.T + b_proj[None, :]
    return final



# revision 6
# speedup vs baseline: 1.1860x; 1.1860x over previous
"""FAVOR+ attention (Performer) Trainium2 kernel, 8-way sharded.

Sharding: 8 cores = 4 batches x 2 head-groups. Core c handles batch c//2 and
heads [8*(c%2), 8*(c%2)+8). The attention core (kv state) is fully local per
head; the output projection is computed as a per-core partial over its 512
input channels and the two partials per batch are summed on the host.

All three input GEMMs (q-features, k-features, v) run in fp8e4 DoubleRow
(2 rows/cell -> half the matmuls of bf16); x is quantized once on the host
and kept resident in SBUF for both passes. The attention-out GEMM is folded
into the projection on device: wkvp = blockdiag(kv)^T-free form kv @ Wp plus
the rank-1 v-bias term ksum (x) (bv@Wp), so pass B is just
q' -> normalize -> one fp16 GEMM against wkvp. fp16 (not bf16) is used for
all 16-bit intermediates (same PE speed, 8x finer mantissa).
"""

import numpy as np
import ml_dtypes

import concourse.mybir as mybir
import concourse.tile as tile
from concourse import bacc
from concourse.bass_utils import run_bass_kernel_spmd

F32 = mybir.dt.float32
F16 = mybir.dt.float16
FP8 = mybir.dt.float8e4
AF = mybir.ActivationFunctionType
ALU = mybir.AluOpType
DR = mybir.MatmulPerfMode.DoubleRow

N = 4096
D = 1024
HD = 64
NF = 64
EPS = 1e-4
BLK = 512  # n-block
NBLK = N // BLK
NCH = BLK // 128  # 128-row chunks per block
SCALE = float(HD) ** -0.25
WS = 128.0  # fp8 weight pre-scale (undone on the way out of PSUM)


def _build_nc():
    nc = bacc.Bacc("TRN2", target_bir_lowering=False, debug=False, num_devices=8)

    xt8 = nc.dram_tensor("xt8", [NBLK, 128, 8, BLK], FP8, kind="ExternalInput").ap()
    wqp8 = nc.dram_tensor("wqp8", [128, 8, 512], FP8, kind="ExternalInput").ap()
    wkp8 = nc.dram_tensor("wkp8", [128, 8, 512], FP8, kind="ExternalInput").ap()
    wv8 = nc.dram_tensor("wv8", [128, 8, 512], FP8, kind="ExternalInput").ap()
    wp = nc.dram_tensor("wp", [128, 4, 1024], F16, kind="ExternalInput").ap()
    bqpe = nc.dram_tensor("bqpe", [128, 4], F32, kind="ExternalInput").ap()
    bkpb = nc.dram_tensor("bkpb", [128, 8, 64], F16, kind="ExternalInput").ap()
    bpvb = nc.dram_tensor("bpvb", [128, 4, 1024], F16, kind="ExternalInput").ap()
    ident = nc.dram_tensor("ident", [128, 128], F16, kind="ExternalInput").ap()
    out = nc.dram_tensor("out", [D, N], F16, kind="ExternalOutput").ap()

    out_v = out.rearrange("(oc p) n -> p oc n", p=128)  # [128, 8, 4096]

    with tile.TileContext(nc) as tc:
        with (
            tc.tile_pool(name="consts", bufs=1) as consts,
            tc.tile_pool(name="x8p", bufs=NBLK) as x8p,
            tc.tile_pool(name="work", bufs=2) as work,
            tc.tile_pool(name="small", bufs=4) as small,
            tc.tile_pool(name="pbig", bufs=6, space="PSUM") as pbig,
        ):
            pkv = tc.alloc_tile_pool(name="pkv", bufs=1, space="PSUM")
            # ---- pass-A-critical loads first ----
            wkp8_sb = consts.tile([128, 8, 512], FP8, name="wkp8_sb")
            nc.scalar.dma_start(wkp8_sb[:], wkp8)
            bkpb_sb = consts.tile([128, 8, 64], F16, name="bkpb_sb")
            nc.scalar.dma_start(bkpb_sb[:], bkpb)
            wv8_sb = consts.tile([128, 8, 512], FP8, name="wv8_sb")
            nc.scalar.dma_start(wv8_sb[:], wv8)
            ident_sb = consts.tile([128, 128], F16, name="ident_sb")
            nc.scalar.dma_start(ident_sb[:], ident)
            eps_sb = consts.tile([128, 1], F32, name="eps_sb")
            nc.vector.memset(eps_sb[:], EPS)
            wtmp = consts.tile([128, 512], F16, name="wtmp")
            nc.vector.memset(wtmp[:], 0.125)

            # declared now, loaded during pass A
            wqp8_sb = consts.tile([128, 8, 512], FP8, name="wqp8_sb")
            wp_sb = consts.tile([128, 4, 1024], F16, name="wp_sb")
            bqpe_sb = consts.tile([128, 4], F32, name="bqpe_sb")
            bpvb_sb = consts.tile([128, 4, 1024], F16, name="bpvb_sb")

            # kv accumulators: pairs (0,1) in kvacc0, (2,3) in kvacc1.
            # Layout per pair: 129 cols (64 v-head0 | 64 v-head1 | ksum), stride 130.
            kvacc = [
                pkv.tile([128, 260], F32, name=f"kvacc{t}", tag=f"kvacc{t}")
                for t in range(2)
            ]

            # PE warmup: keep the HAM activity window busy while the first
            # DMAs stream, so real matmuls start at 2.4 GHz.
            ps_warm = pbig.tile([128, 512], F32, name="ps_warm", tag="big")
            for _ in range(14):
                nc.tensor.matmul(
                    ps_warm[:], wtmp[:, 0:128], wtmp[:], start=True, stop=True
                )

            # ================= pass A: k', v -> kv, ksum =================
            def load_x8(blk):
                t = x8p.tile([128, 8, BLK], FP8, name="x8_t", tag="x8")
                nc.sync.dma_start(t[:], xt8[blk])
                return t

            def emit_v(x8_t, c, v_sbs):
                cs = slice(c * 128, (c + 1) * 128)
                psv = pbig.tile([128, 512], F32, name="ps_v", tag="big")
                for k in range(4):
                    nc.tensor.matmul(
                        psv[:],
                        x8_t[:, 2 * k : 2 * k + 2, cs],
                        wv8_sb[:, 2 * k : 2 * k + 2, :],
                        start=(k == 0),
                        stop=(k == 3),
                        perf_mode=DR,
                    )
                v_sb = work.tile([128, 4, 132], F16, name="v_sb", tag="v", bufs=5)
                nc.scalar.activation(
                    v_sb[:, :, 0:128],
                    psv.rearrange("p (g j) -> p g j", j=128),
                    AF.Copy,
                    scale=1.0 / WS,
                )
                nc.vector.memset(v_sb[:, :, 128:129], 1.0)
                v_sbs.append(v_sb)

            def emit_kf(x8_t, c, kp_sbs):
                cs = slice(c * 128, (c + 1) * 128)
                psf = pbig.tile([128, 512], F32, name="ps_kf", tag="big")
                for k in range(4):
                    nc.tensor.matmul(
                        psf[:],
                        x8_t[:, 2 * k : 2 * k + 2, cs],
                        wkp8_sb[:, 2 * k : 2 * k + 2, :],
                        start=(k == 0),
                        stop=(k == 3),
                        perf_mode=DR,
                    )
                psf_v = psf.rearrange("p (g f) -> p g f", f=64)  # [128, 8, 64]
                karg = small.tile([128, 8, 64], F16, name="karg", tag="karg")
                nc.vector.tensor_tensor(karg[:], psf_v, bkpb_sb[:], ALU.add)
                mx = small.tile([128, 8], F32, name="mx", tag="mx")
                nc.vector.reduce_max(mx[:], karg[:], axis=mybir.AxisListType.X)
                nc.gpsimd.tensor_tensor(
                    karg[:], karg[:],
                    mx[:, :, None].to_broadcast([128, 8, 64]),
                    ALU.subtract,
                )
                kp_sb = work.tile([128, 4, 128], F16, name="kp_sb", tag="kp", bufs=9)
                nc.scalar.activation(
                    kp_sb.rearrange("p g (h f) -> p (g h) f", f=64),
                    karg[:], AF.Exp, bias=eps_sb[:], scale=1.0 / WS,
                )
                kp_sbs.append(kp_sb)

            def emit_kv(blk, c, kp_sbs, v_sbs):
                glob_first = blk == 0 and c == 0
                glob_last = blk == NBLK - 1 and c == NCH - 1
                for p in range(4):
                    base = (p % 2) * 130
                    nc.tensor.matmul(
                        kvacc[p // 2][:, base : base + 129],
                        kp_sbs[c][:, p, :],
                        v_sbs[c][:, p, 0:129],
                        start=(glob_first and p % 2 == 0),
                        stop=(glob_last and p % 2 == 1),
                    )

            x8_blks = [load_x8(0), load_x8(1)]
            for blk in range(NBLK):
                if blk + 2 < NBLK:
                    x8_blks.append(load_x8(blk + 2))
                if blk == 2:
                    # stream pass-B constants once startup DMA pressure is
                    # over (gpsimd SWDGE; sync keeps feeding xt8 blocks)
                    nc.gpsimd.dma_start(wqp8_sb[:], wqp8)
                    nc.gpsimd.dma_start(wp_sb[:], wp)
                    nc.gpsimd.dma_start(bqpe_sb[:], bqpe)
                    nc.gpsimd.dma_start(bpvb_sb[:], bpvb)
                x8_t = x8_blks[blk]
                v_sbs, kp_sbs = [], []
                for c in range(NCH):
                    emit_kf(x8_t, c, kp_sbs)
                    emit_v(x8_t, c, v_sbs)
                for c in range(NCH):
                    emit_kv(blk, c, kp_sbs, v_sbs)

            # ======== boundary: ksum columns + wkvp = kv @ Wp fold ========
            # Pre-queue the first two blocks' q-feature GEMMs so the PE has
            # work while DVE/ACT assemble ksbc/wkvp.
            def emit_qp(blk):
                x8_t = x8_blks[blk]
                qp_sb = work.tile([128, 4, BLK], F16, name="qp_sb", tag="qp", bufs=4)
                for p in range(4):
                    ps = pbig.tile([128, BLK], F32, name="ps_qt", tag="big")
                    for k in range(4):
                        nc.tensor.matmul(
                            ps[:],
                            wqp8_sb[:, 2 * k : 2 * k + 2, p * 128 : (p + 1) * 128],
                            x8_t[:, 2 * k : 2 * k + 2, :],
                            start=(k == 0),
                            stop=(k == 3),
                            perf_mode=DR,
                        )
                    nc.scalar.activation(
                        qp_sb[:, p, :], ps[:], AF.Exp,
                        bias=bqpe_sb[:, p : p + 1], scale=1.0 / WS,
                    )
                return qp_sb

            qps = [emit_qp(0), emit_qp(1)]

            # ksbc_ext[hf, p, j]: ksum[hf] masked to head(j)'s block, the
            # stationary operand of the merged normalizer+broadcast matmul.
            ksbc = consts.tile([128, 4, 128], F16, name="ksbc")
            nc.vector.memset(ksbc[:], 0.0)
            for p in range(4):
                t = kvacc[p // 2]
                base = (p % 2) * 130
                ks = t[:, base + 128 : base + 129]
                nc.vector.tensor_copy(
                    out=ksbc[0:64, p, 0:64], in_=ks[0:64].to_broadcast([64, 64])
                )
                nc.vector.tensor_copy(
                    out=ksbc[64:128, p, 64:128], in_=ks[64:128].to_broadcast([64, 64])
                )

            # wkvp[hf, p, od] = sum_{vd in head(hf)} kv[hf,vd] Wp[vd,od]
            #                   + ksum[hf] * (bv @ Wp)[head(hf), od]
            wkvp_sb = consts.tile([128, 4, 1024], F16, name="wkvp_sb")
            for p in range(4):
                t = kvacc[p // 2]
                base = (p % 2) * 130
                ks = t[:, base + 128 : base + 129]
                # block-diagonal extract (off-diag quadrants are cross-head
                # garbage from the full outer-product accumulation)
                kvsb = small.tile([128, 128], F16, name="kvsb", tag="kvsb", bufs=2)
                nc.vector.memset(kvsb[:], 0.0)
                nc.vector.tensor_copy(out=kvsb[0:64, 0:64], in_=t[0:64, base : base + 64])
                nc.vector.tensor_copy(
                    out=kvsb[64:128, 64:128], in_=t[64:128, base + 64 : base + 128]
                )
                psT = pbig.tile([128, 128], F16, name="ps_T", tag="big")
                nc.tensor.transpose(psT[:], kvsb[:], ident_sb[:])
                kvT = small.tile([128, 128], F16, name="kvT", tag="kvT", bufs=2)
                nc.scalar.copy(kvT[:], psT[:])
                for half in range(2):
                    hs = slice(half * 512, (half + 1) * 512)
                    pw = pbig.tile([128, 512], F32, name="ps_w", tag="big")
                    nc.tensor.matmul(
                        pw[:], kvT[:], wp_sb[:, p, hs], start=True, stop=True
                    )
                    nc.vector.scalar_tensor_tensor(
                        out=wkvp_sb[:, p, hs],
                        in0=bpvb_sb[:, p, hs],
                        scalar=ks,
                        in1=pw[:],
                        op0=ALU.mult,
                        op1=ALU.add,
                    )

            pkv.release()
            pnrm = tc.alloc_tile_pool(name="pnrm", bufs=2, space="PSUM")

            # ================= pass B: q' -> normalize -> proj =================
            def emit_pj(blk, q2s, oc_range):
                ns = slice(blk * BLK, (blk + 1) * BLK)
                for oc in oc_range:
                    pj = pbig.tile([128, BLK], F32, name="ps_pj", tag="big")
                    for p in range(4):
                        nc.tensor.matmul(
                            pj[:],
                            wkvp_sb[:, p, oc * 128 : (oc + 1) * 128],
                            q2s[p][:],
                            start=(p == 0),
                            stop=(p == 3),
                        )
                    pj_sb = small.tile([128, BLK], F16, name="pj_sb", tag="pj", bufs=6)
                    if oc % 2 == 0:
                        nc.vector.tensor_copy(out=pj_sb[:], in_=pj[:])
                        nc.sync.dma_start(out_v[:, oc, ns], pj_sb[:])
                    else:
                        nc.scalar.copy(pj_sb[:], pj[:])
                        nc.scalar.dma_start(out_v[:, oc, ns], pj_sb[:])

            prev_q2 = None  # (blk, q2s) whose proj is still pending
            for blk in range(NBLK):
                qp_sb = qps[blk]
                # merged normalizer+broadcast: one matmul per pair gives the
                # per-head norm already broadcast over its 64 partitions
                q2s = []
                for p in range(4):
                    nrm = pnrm.tile([128, BLK], F32, name="nrm", tag="nrm")
                    nc.tensor.matmul(
                        nrm[:], ksbc[:, p, :], qp_sb[:, p, :], start=True, stop=True
                    )
                    rec = small.tile([128, BLK], F32, name="rec", tag="rec", bufs=3)
                    nc.vector.reciprocal_approx_fast(out=rec[:], in_=nrm[:])
                    q2 = small.tile([128, BLK], F16, name="q2", tag="q2", bufs=9)
                    nc.vector.tensor_mul(q2[:], qp_sb[:, p, :], rec[:])
                    q2s.append(q2)

                if blk + 2 < NBLK:
                    qps.append(emit_qp(blk + 2))
                if prev_q2 is not None:
                    emit_pj(prev_q2[0], prev_q2[1], range(8))
                prev_q2 = (blk, q2s)

            emit_pj(prev_q2[0], prev_q2[1], range(8))

            pnrm.release()

    nc.compile()
    return nc


_NC = None


def _get_nc():
    global _NC
    if _NC is None:
        _NC = _build_nc()
    return _NC


def _host_inputs(x, W_qkv, b_qkv, W_proj, b_proj, proj_mat):
    x = np.asarray(x, dtype=np.float32)
    W_qkv = np.asarray(W_qkv, dtype=np.float32)
    b_qkv = np.asarray(b_qkv, dtype=np.float32)
    W_proj = np.asarray(W_proj, dtype=np.float32)
    proj_mat = np.asarray(proj_mat, dtype=np.float32)

    pt = (proj_mat.T * SCALE).astype(np.float32)  # [hd, F]

    def tile_x(xb):
        # [NBLK, 128, 8, BLK]: contiguous per-partition runs for fast DMA
        xt = xb.T.reshape(8, 128, NBLK, BLK).transpose(2, 1, 0, 3)
        return np.ascontiguousarray(xt)

    def tile_w(w):
        # [D, 512] -> [128, 8, 512]
        return np.ascontiguousarray(w.reshape(8, 128, 512).transpose(1, 0, 2))

    xt8s = [tile_x(x[b]).astype(ml_dtypes.float8_e4m3) for b in range(4)]
    ident = np.eye(128, dtype=np.float16)

    def fuse(Wslc, bslc):
        # W_fused[:, (h f)] = sum_d W.T[:, (h d)] pt[d, f]; bias likewise
        wT = Wslc.T.reshape(D, 8, HD)
        wf = np.einsum("ahd,df->ahf", wT, pt).reshape(D, 512)
        bf = np.einsum("hd,df->hf", bslc.reshape(8, HD), pt).reshape(512)
        return wf, bf

    in_maps = []
    for c in range(8):
        b, g = c // 2, c % 2
        wqs = W_qkv[g * 512 : (g + 1) * 512]
        wks = W_qkv[D + g * 512 : D + (g + 1) * 512]
        wvs = W_qkv[2 * D + g * 512 : 2 * D + (g + 1) * 512]
        bqs = b_qkv[g * 512 : (g + 1) * 512]
        bks = b_qkv[D + g * 512 : D + (g + 1) * 512]
        bvs = b_qkv[2 * D + g * 512 : 2 * D + (g + 1) * 512]
        wqp, bqp = fuse(wqs, bqs)
        wkp, bkp = fuse(wks, bks)
        wp_loc = W_proj[:, g * 512 : (g + 1) * 512].T  # [512 vd, 1024 od]
        # rank-1 v-bias fold: bpv_h = bv_h @ Wp rows of head h
        bpvb = np.empty((128, 4, 1024), dtype=np.float16)
        for p in range(4):
            for hh in range(2):
                vd = slice(p * 128 + hh * 64, p * 128 + hh * 64 + 64)
                row = bvs[vd] @ wp_loc[vd]  # [1024]
                bpvb[hh * 64 : hh * 64 + 64, p, :] = row[None, :]
        in_maps.append(
            {
                "xt8": xt8s[b],
                "wqp8": tile_w(wqp * WS).astype(ml_dtypes.float8_e4m3),
                "wkp8": tile_w(wkp * WS).astype(ml_dtypes.float8_e4m3),
                "wv8": tile_w(np.ascontiguousarray(wvs.T) * WS).astype(
                    ml_dtypes.float8_e4m3
                ),
                "wp": np.ascontiguousarray(
                    wp_loc.reshape(4, 128, 1024).transpose(1, 0, 2)
                ).astype(np.float16),
                "bqpe": np.ascontiguousarray(
                    (bqp + EPS).reshape(4, 128).T
                ).astype(np.float32),
                "bkpb": np.ascontiguousarray(
                    np.broadcast_to(bkp.reshape(1, 8, 64) * WS, (128, 8, 64))
                ).astype(np.float16),
                "bpvb": bpvb,
                "ident": ident,
            }
        )
    return in_maps


def kernel(x, W_qkv, b_qkv, W_proj, b_proj, proj_mat):
    b_proj = np.asarray(b_proj, dtype=np.float32)
    in_maps = _host_inputs(x, W_qkv, b_qkv, W_proj, b_proj, proj_mat)
    nc = _get_nc()
    res = run_bass_kernel_spmd(nc, in_maps, core_ids=list(range(8)))
    final = np.empty((4, N, D), dtype=np.float32)
    for b in range(4):
        acc = res.results[2 * b]["out"].astype(np.float32) + res.results[
            2 * b + 1
        ]["out"].astype(np.float32)
        final[b] = acc.T + b_proj[None, :]
    return final


# revision 12
# speedup vs baseline: 1.2455x; 1.0502x over previous
"""FAVOR+ attention (Performer) Trainium2 kernel, 8-way sharded.

Sharding: 8 cores = 4 batches x 2 head-groups. Core c handles batch c//2 and
heads [8*(c%2), 8*(c%2)+8). The attention core (kv state) is fully local per
head; the output projection is computed as a per-core partial over its 512
input channels and the two partials per batch are summed on the host.

All three input GEMMs (q-features, k-features, v) run in fp8e4 DoubleRow
(2 rows/cell -> half the matmuls of bf16); x is quantized once on the host
and kept resident in SBUF for both passes. The attention-out GEMM is folded
into the projection on device: wkvp = blockdiag(kv)^T-free form kv @ Wp plus
the rank-1 v-bias term ksum (x) (bv@Wp), so pass B is just
q' -> normalize -> one fp16 GEMM against wkvp. fp16 (not bf16) is used for
all 16-bit intermediates (same PE speed, 8x finer mantissa).
"""

import numpy as np
import ml_dtypes

import concourse.mybir as mybir
import concourse.tile as tile
from concourse import bacc
from concourse.bass_utils import run_bass_kernel_spmd

F32 = mybir.dt.float32
F16 = mybir.dt.float16
FP8 = mybir.dt.float8e4
AF = mybir.ActivationFunctionType
ALU = mybir.AluOpType
DR = mybir.MatmulPerfMode.DoubleRow

N = 4096
D = 1024
HD = 64
NF = 64
EPS = 1e-4
BLK = 512  # n-block
NBLK = N // BLK
NCH = BLK // 128  # 128-row chunks per block
SCALE = float(HD) ** -0.25
WS = 128.0  # fp8 weight pre-scale (undone on the way out of PSUM)


def _build_nc():
    nc = bacc.Bacc("TRN2", target_bir_lowering=False, debug=False, num_devices=8)

    xt8 = nc.dram_tensor("xt8", [NBLK, 128, 8, BLK], FP8, kind="ExternalInput").ap()
    wqp8 = nc.dram_tensor("wqp8", [128, 8, 512], FP8, kind="ExternalInput").ap()
    wkp8 = nc.dram_tensor("wkp8", [128, 8, 512], FP8, kind="ExternalInput").ap()
    wv8 = nc.dram_tensor("wv8", [128, 8, 512], FP8, kind="ExternalInput").ap()
    wp = nc.dram_tensor("wp", [128, 4, 1024], F16, kind="ExternalInput").ap()
    bqpe = nc.dram_tensor("bqpe", [128, 4], F32, kind="ExternalInput").ap()
    bkpb = nc.dram_tensor("bkpb", [128, 8, 64], F16, kind="ExternalInput").ap()
    bpvb = nc.dram_tensor("bpvb", [128, 4, 1024], F16, kind="ExternalInput").ap()
    ident = nc.dram_tensor("ident", [128, 128], F16, kind="ExternalInput").ap()
    out = nc.dram_tensor("out", [D, N], F16, kind="ExternalOutput").ap()

    out_v = out.rearrange("(oc p) n -> p oc n", p=128)  # [128, 8, 4096]

    with tile.TileContext(nc) as tc:
        with (
            tc.tile_pool(name="consts", bufs=1) as consts,
            tc.tile_pool(name="x8p", bufs=NBLK) as x8p,
            tc.tile_pool(name="work", bufs=2) as work,
            tc.tile_pool(name="small", bufs=4) as small,
            tc.tile_pool(name="pbig", bufs=6, space="PSUM") as pbig,
        ):
            pkv = tc.alloc_tile_pool(name="pkv", bufs=1, space="PSUM")
            # ---- pass-A-critical loads first ----
            wkp8_sb = consts.tile([128, 8, 512], FP8, name="wkp8_sb")
            nc.scalar.dma_start(wkp8_sb[:], wkp8)
            bkpb_sb = consts.tile([128, 8, 64], F16, name="bkpb_sb")
            nc.scalar.dma_start(bkpb_sb[:], bkpb)
            wv8_sb = consts.tile([128, 8, 512], FP8, name="wv8_sb")
            nc.scalar.dma_start(wv8_sb[:], wv8)
            ident_sb = consts.tile([128, 128], F16, name="ident_sb")
            nc.scalar.dma_start(ident_sb[:], ident)
            eps_sb = consts.tile([128, 1], F32, name="eps_sb")
            nc.vector.memset(eps_sb[:], EPS)
            wtmp = consts.tile([128, 512], F16, name="wtmp")
            nc.vector.memset(wtmp[:], 0.125)

            # pass-B constants ride the same scalar ring AFTER the pass-A
            # critical set — ring FIFO keeps them from stealing bandwidth
            # from the transfers the first matmuls block on.
            wqp8_sb = consts.tile([128, 8, 512], FP8, name="wqp8_sb")
            nc.scalar.dma_start(wqp8_sb[:], wqp8)
            wp_sb = consts.tile([128, 4, 1024], F16, name="wp_sb")
            nc.scalar.dma_start(wp_sb[:], wp)
            bqpe_sb = consts.tile([128, 4], F32, name="bqpe_sb")
            nc.scalar.dma_start(bqpe_sb[:], bqpe)
            bpvb_sb = consts.tile([128, 4, 1024], F16, name="bpvb_sb")
            nc.scalar.dma_start(bpvb_sb[:], bpvb)

            # kv accumulators: pairs (0,1) in kvacc0, (2,3) in kvacc1.
            # Layout per pair: 129 cols (64 v-head0 | 64 v-head1 | ksum), stride 130.
            kvacc = [
                pkv.tile([128, 260], F32, name=f"kvacc{t}", tag=f"kvacc{t}")
                for t in range(2)
            ]

            # PE warmup: keep the HAM activity window busy while the first
            # DMAs stream, so real matmuls start at 2.4 GHz.
            ps_warm = pbig.tile([128, 512], F32, name="ps_warm", tag="big")
            for _ in range(16):
                nc.tensor.matmul(
                    ps_warm[:, 0:128], wtmp[:, 0:128], wtmp[:, 0:128],
                    start=True, stop=True,
                )

            # ================= pass A: k', v -> kv, ksum =================
            def load_x8(blk):
                t = x8p.tile([128, 8, BLK], FP8, name="x8_t", tag="x8")
                nc.sync.dma_start(t[:], xt8[blk])
                return t

            def emit_v(x8_t, c, v_sbs):
                cs = slice(c * 128, (c + 1) * 128)
                psv = pbig.tile([128, 512], F32, name="ps_v", tag="big")
                for k in range(4):
                    nc.tensor.matmul(
                        psv[:],
                        x8_t[:, 2 * k : 2 * k + 2, cs],
                        wv8_sb[:, 2 * k : 2 * k + 2, :],
                        start=(k == 0),
                        stop=(k == 3),
                        perf_mode=DR,
                    )
                v_sb = work.tile([128, 4, 132], F16, name="v_sb", tag="v", bufs=5)
                nc.scalar.activation(
                    v_sb[:, :, 0:128],
                    psv.rearrange("p (g j) -> p g j", j=128),
                    AF.Copy,
                    scale=1.0 / WS,
                )
                nc.vector.memset(v_sb[:, :, 128:129], 1.0)
                v_sbs.append(v_sb)

            def emit_kf(x8_t, c, kp_sbs):
                cs = slice(c * 128, (c + 1) * 128)
                psf = pbig.tile([128, 512], F32, name="ps_kf", tag="big")
                for k in range(4):
                    nc.tensor.matmul(
                        psf[:],
                        x8_t[:, 2 * k : 2 * k + 2, cs],
                        wkp8_sb[:, 2 * k : 2 * k + 2, :],
                        start=(k == 0),
                        stop=(k == 3),
                        perf_mode=DR,
                    )
                psf_v = psf.rearrange("p (g f) -> p g f", f=64)  # [128, 8, 64]
                karg = small.tile([128, 8, 64], F16, name="karg", tag="karg")
                nc.vector.tensor_tensor(karg[:], psf_v, bkpb_sb[:], ALU.add)
                mx = small.tile([128, 8], F32, name="mx", tag="mx")
                nc.vector.reduce_max(mx[:], karg[:], axis=mybir.AxisListType.X)
                nc.gpsimd.tensor_tensor(
                    karg[:], karg[:],
                    mx[:, :, None].to_broadcast([128, 8, 64]),
                    ALU.subtract,
                )
                kp_sb = work.tile([128, 4, 128], F16, name="kp_sb", tag="kp", bufs=9)
                nc.scalar.activation(
                    kp_sb.rearrange("p g (h f) -> p (g h) f", f=64),
                    karg[:], AF.Exp, bias=eps_sb[:], scale=1.0 / WS,
                )
                kp_sbs.append(kp_sb)

            def emit_kv(blk, c, kp_sbs, v_sbs):
                glob_first = blk == 0 and c == 0
                glob_last = blk == NBLK - 1 and c == NCH - 1
                for p in range(4):
                    base = (p % 2) * 130
                    nc.tensor.matmul(
                        kvacc[p // 2][:, base : base + 129],
                        kp_sbs[c][:, p, :],
                        v_sbs[c][:, p, 0:129],
                        start=(glob_first and p % 2 == 0),
                        stop=(glob_last and p % 2 == 1),
                    )

            # blocks 0+1: k-features first (only need wkp8 + x8), so the PE
            # has work while wv8 is still streaming on the scalar ring.
            x8_blks = [load_x8(0), load_x8(1), load_x8(2), load_x8(3)]
            kp01 = [[], []]
            v01 = [[], []]
            for b in range(2):
                for c in range(NCH):
                    emit_kf(x8_blks[b], c, kp01[b])
            for b in range(2):
                for c in range(NCH):
                    emit_v(x8_blks[b], c, v01[b])
                for c in range(NCH):
                    emit_kv(b, c, kp01[b], v01[b])

            for blk in range(2, NBLK):
                if blk + 2 < NBLK:
                    x8_blks.append(load_x8(blk + 2))
                x8_t = x8_blks[blk]
                v_sbs, kp_sbs = [], []
                for c in range(NCH):
                    emit_kf(x8_t, c, kp_sbs)
                    emit_v(x8_t, c, v_sbs)
                for c in range(NCH):
                    emit_kv(blk, c, kp_sbs, v_sbs)

            # ======== boundary: ksum columns + wkvp = kv @ Wp fold ========
            # Pre-queue the first two blocks' q-feature GEMMs so the PE has
            # work while DVE/ACT assemble ksbc/wkvp.
            def emit_qp(blk):
                x8_t = x8_blks[blk]
                qp_sb = work.tile([128, 4, BLK], F16, name="qp_sb", tag="qp", bufs=6)
                for p in range(4):
                    ps = pbig.tile([128, BLK], F32, name="ps_qt", tag="big")
                    for k in range(4):
                        nc.tensor.matmul(
                            ps[:],
                            wqp8_sb[:, 2 * k : 2 * k + 2, p * 128 : (p + 1) * 128],
                            x8_t[:, 2 * k : 2 * k + 2, :],
                            start=(k == 0),
                            stop=(k == 3),
                            perf_mode=DR,
                        )
                    nc.scalar.activation(
                        qp_sb[:, p, :], ps[:], AF.Exp,
                        bias=bqpe_sb[:, p : p + 1], scale=1.0 / WS,
                    )
                return qp_sb

            qps = [emit_qp(0), emit_qp(1), emit_qp(2), emit_qp(3)]

            # ksbc_ext[hf, p, j]: ksum[hf] masked to head(j)'s block, the
            # stationary operand of the merged normalizer+broadcast matmul.
            ksbc = consts.tile([128, 4, 128], F16, name="ksbc")
            nc.vector.memset(ksbc[:], 0.0)
            for p in range(4):
                t = kvacc[p // 2]
                base = (p % 2) * 130
                ks = t[:, base + 128 : base + 129]
                nc.vector.tensor_copy(
                    out=ksbc[0:64, p, 0:64], in_=ks[0:64].to_broadcast([64, 64])
                )
                nc.vector.tensor_copy(
                    out=ksbc[64:128, p, 64:128], in_=ks[64:128].to_broadcast([64, 64])
                )

            # wkvp[hf, p, od] = sum_{vd in head(hf)} kv[hf,vd] Wp[vd,od]
            #                   + ksum[hf] * (bv @ Wp)[head(hf), od]
            wkvp_sb = consts.tile([128, 4, 1024], F16, name="wkvp_sb")
            for p in range(4):
                t = kvacc[p // 2]
                base = (p % 2) * 130
                ks = t[:, base + 128 : base + 129]
                # block-diagonal extract (off-diag quadrants are cross-head
                # garbage from the full outer-product accumulation)
                kvsb = small.tile([128, 128], F16, name="kvsb", tag="kvsb", bufs=2)
                nc.vector.memset(kvsb[:], 0.0)
                nc.vector.tensor_copy(out=kvsb[0:64, 0:64], in_=t[0:64, base : base + 64])
                nc.vector.tensor_copy(
                    out=kvsb[64:128, 64:128], in_=t[64:128, base + 64 : base + 128]
                )
                psT = pbig.tile([128, 128], F16, name="ps_T", tag="big")
                nc.tensor.transpose(psT[:], kvsb[:], ident_sb[:])
                kvT = small.tile([128, 128], F16, name="kvT", tag="kvT", bufs=2)
                nc.scalar.copy(kvT[:], psT[:])
                for half in range(2):
                    hs = slice(half * 512, (half + 1) * 512)
                    pw = pbig.tile([128, 512], F32, name="ps_w", tag="big")
                    nc.tensor.matmul(
                        pw[:], kvT[:], wp_sb[:, p, hs], start=True, stop=True
                    )
                    nc.vector.scalar_tensor_tensor(
                        out=wkvp_sb[:, p, hs],
                        in0=bpvb_sb[:, p, hs],
                        scalar=ks,
                        in1=pw[:],
                        op0=ALU.mult,
                        op1=ALU.add,
                    )

            pkv.release()
            pnrm = tc.alloc_tile_pool(name="pnrm", bufs=2, space="PSUM")

            # ================= pass B: q' -> normalize -> proj =================
            def emit_pj(blk, q2s, oc_range):
                ns = slice(blk * BLK, (blk + 1) * BLK)
                for oc in oc_range:
                    pj = pbig.tile([128, BLK], F32, name="ps_pj", tag="big")
                    for p in range(4):
                        nc.tensor.matmul(
                            pj[:],
                            wkvp_sb[:, p, oc * 128 : (oc + 1) * 128],
                            q2s[p][:],
                            start=(p == 0),
                            stop=(p == 3),
                        )
                    pj_sb = small.tile([128, BLK], F16, name="pj_sb", tag="pj", bufs=6)
                    if oc % 2 == 0:
                        nc.vector.tensor_copy(out=pj_sb[:], in_=pj[:])
                    else:
                        nc.scalar.copy(pj_sb[:], pj[:])
                    eng = (nc.sync, nc.scalar, nc.gpsimd)[oc % 3]
                    eng.dma_start(out_v[:, oc, ns], pj_sb[:])

            def emit_nrm(blk):
                # merged normalizer+broadcast: one matmul per pair gives the
                # per-head norm already broadcast over its 64 partitions
                qp_sb = qps[blk]
                q2s = []
                for p in range(4):
                    nrm = pnrm.tile([128, BLK], F32, name="nrm", tag="nrm")
                    nc.tensor.matmul(
                        nrm[:], ksbc[:, p, :], qp_sb[:, p, :], start=True, stop=True
                    )
                    rec = small.tile([128, BLK], F32, name="rec", tag="rec", bufs=3)
                    nc.vector.reciprocal_approx_fast(out=rec[:], in_=nrm[:])
                    q2 = small.tile([128, BLK], F16, name="q2", tag="q2", bufs=9)
                    nc.vector.tensor_mul(q2[:], qp_sb[:, p, :], rec[:])
                    q2s.append(q2)
                return q2s

            prev_q2 = None  # (blk, q2s) whose proj is still pending
            for blk in range(NBLK - 1):
                q2s = emit_nrm(blk)
                if blk + 4 < NBLK:
                    qps.append(emit_qp(blk + 4))
                if prev_q2 is not None:
                    emit_pj(prev_q2[0], prev_q2[1], range(8))
                prev_q2 = (blk, q2s)

            # last block: interleave the two pending projs so the final
            # out-DMAs start ~4us earlier and the DMA rings drain in time
            q2s7 = emit_nrm(NBLK - 1)
            emit_pj(prev_q2[0], prev_q2[1], range(0, 4))
            emit_pj(NBLK - 1, q2s7, range(0, 2))
            emit_pj(prev_q2[0], prev_q2[1], range(4, 8))
            emit_pj(NBLK - 1, q2s7, range(2, 6))
            emit_pj(NBLK - 1, q2s7, range(6, 8))

            pnrm.release()

    nc.compile()
    return nc


_NC = None


def _get_nc():
    global _NC
    if _NC is None:
        _NC = _build_nc()
    return _NC


def _host_inputs(x, W_qkv, b_qkv, W_proj, b_proj, proj_mat):
    x = np.asarray(x, dtype=np.float32)
    W_qkv = np.asarray(W_qkv, dtype=np.float32)
    b_qkv = np.asarray(b_qkv, dtype=np.float32)
    W_proj = np.asarray(W_proj, dtype=np.float32)
    proj_mat = np.asarray(proj_mat, dtype=np.float32)

    pt = (proj_mat.T * SCALE).astype(np.float32)  # [hd, F]

    def tile_x(xb):
        # [NBLK, 128, 8, BLK]: contiguous per-partition runs for fast DMA
        xt = xb.T.reshape(8, 128, NBLK, BLK).transpose(2, 1, 0, 3)
        return np.ascontiguousarray(xt)

    def tile_w(w):
        # [D, 512] -> [128, 8, 512]
        return np.ascontiguousarray(w.reshape(8, 128, 512).transpose(1, 0, 2))

    xt8s = [tile_x(x[b]).astype(ml_dtypes.float8_e4m3) for b in range(4)]
    ident = np.eye(128, dtype=np.float16)

    def fuse(Wslc, bslc):
        # W_fused[:, (h f)] = sum_d W.T[:, (h d)] pt[d, f]; bias likewise
        wT = Wslc.T.reshape(D, 8, HD)
        wf = np.einsum("ahd,df->ahf", wT, pt).reshape(D, 512)
        bf = np.einsum("hd,df->hf", bslc.reshape(8, HD), pt).reshape(512)
        return wf, bf

    in_maps = []
    for c in range(8):
        b, g = c // 2, c % 2
        wqs = W_qkv[g * 512 : (g + 1) * 512]
        wks = W_qkv[D + g * 512 : D + (g + 1) * 512]
        wvs = W_qkv[2 * D + g * 512 : 2 * D + (g + 1) * 512]
        bqs = b_qkv[g * 512 : (g + 1) * 512]
        bks = b_qkv[D + g * 512 : D + (g + 1) * 512]
        bvs = b_qkv[2 * D + g * 512 : 2 * D + (g + 1) * 512]
        wqp, bqp = fuse(wqs, bqs)
        wkp, bkp = fuse(wks, bks)
        wp_loc = W_proj[:, g * 512 : (g + 1) * 512].T  # [512 vd, 1024 od]
        # rank-1 v-bias fold: bpv_h = bv_h @ Wp rows of head h
        bpvb = np.empty((128, 4, 1024), dtype=np.float16)
        for p in range(4):
            for hh in range(2):
                vd = slice(p * 128 + hh * 64, p * 128 + hh * 64 + 64)
                row = bvs[vd] @ wp_loc[vd]  # [1024]
                bpvb[hh * 64 : hh * 64 + 64, p, :] = row[None, :]
        in_maps.append(
            {
                "xt8": xt8s[b],
                "wqp8": tile_w(wqp * WS).astype(ml_dtypes.float8_e4m3),
                "wkp8": tile_w(wkp * WS).astype(ml_dtypes.float8_e4m3),
                "wv8": tile_w(np.ascontiguousarray(wvs.T) * WS).astype(
                    ml_dtypes.float8_e4m3
                ),
                "wp": np.ascontiguousarray(
                    wp_loc.reshape(4, 128, 1024).transpose(1, 0, 2)
                ).astype(np.float16),
                "bqpe": np.ascontiguousarray(
                    (bqp + EPS).reshape(4, 128).T
                ).astype(np.float32),
                "bkpb": np.ascontiguousarray(
                    np.broadcast_to(bkp.reshape(1, 8, 64) * WS, (128, 8, 64))
                ).astype(np.float16),
                "bpvb": bpvb,
                "ident": ident,
            }
        )
    return in_maps


def kernel(x, W_qkv, b_qkv, W_proj, b_proj, proj_mat):
    b_proj = np.asarray(b_proj, dtype=np.float32)
    in_maps = _host_inputs(x, W_qkv, b_qkv, W_proj, b_proj, proj_mat)
    nc = _get_nc()
    res = run_bass_kernel_spmd(nc, in_maps, core_ids=list(range(8)))
    final = np.empty((4, N, D), dtype=np.float32)
    for b in range(4):
        acc = res.results[2 * b]["out"].astype(np.float32) + res.results[
            2 * b + 1
        ]["out"].astype(np.float32)
        final[b] = acc.T + b_proj[None, :]
    return final


# revision 23
# speedup vs baseline: 1.3242x; 1.0631x over previous
"""FAVOR+ attention (Performer) Trainium2 kernel, 8-way sharded.

Sharding: 8 cores = 4 batches x 2 head-groups. Core c handles batch c//2 and
heads [8*(c%2), 8*(c%2)+8). The attention core (kv state) is fully local per
head; the output projection is computed as a per-core partial over its 512
input channels and the two partials per batch are summed on the host.

All three input GEMMs (q-features, k-features, v) run in fp8e4 DoubleRow
(2 rows/cell -> half the matmuls of bf16); x is quantized once on the host
and kept resident in SBUF for both passes. The attention-out GEMM is folded
into the projection on device: wkvp = blockdiag(kv)^T-free form kv @ Wp plus
the rank-1 v-bias term ksum (x) (bv@Wp), so pass B is just
q' -> normalize -> one fp16 GEMM against wkvp. fp16 (not bf16) is used for
all 16-bit intermediates (same PE speed, 8x finer mantissa).
"""

import numpy as np
import ml_dtypes

import concourse.mybir as mybir
import concourse.tile as tile
from concourse import bacc
from concourse.bass_utils import run_bass_kernel_spmd

F32 = mybir.dt.float32
F16 = mybir.dt.float16
FP8 = mybir.dt.float8e4
AF = mybir.ActivationFunctionType
ALU = mybir.AluOpType
DR = mybir.MatmulPerfMode.DoubleRow

N = 4096
D = 1024
HD = 64
NF = 64
EPS = 1e-4
BLK = 512  # n-block
NBLK = N // BLK
NCH = BLK // 128  # 128-row chunks per block
SCALE = float(HD) ** -0.25
WS = 128.0  # fp8 weight pre-scale (undone on the way out of PSUM)


def _build_nc():
    nc = bacc.Bacc("TRN2", target_bir_lowering=False, debug=False, num_devices=8)

    xt8 = nc.dram_tensor("xt8", [NBLK, 128, 8, BLK], FP8, kind="ExternalInput").ap()
    wqp8 = nc.dram_tensor("wqp8", [128, 8, 512], FP8, kind="ExternalInput").ap()
    wkp8 = nc.dram_tensor("wkp8", [128, 8, 512], FP8, kind="ExternalInput").ap()
    wv8 = nc.dram_tensor("wv8", [128, 8, 512], FP8, kind="ExternalInput").ap()
    wp = nc.dram_tensor("wp", [128, 4, 1024], F16, kind="ExternalInput").ap()
    bqpe = nc.dram_tensor("bqpe", [128, 4], F32, kind="ExternalInput").ap()
    bkpb = nc.dram_tensor("bkpb", [128, 8, 64], F16, kind="ExternalInput").ap()
    ident = nc.dram_tensor("ident", [128, 128], F16, kind="ExternalInput").ap()
    out = nc.dram_tensor("out", [D, N], F16, kind="ExternalOutput").ap()

    out_v = out.rearrange("(oc p) n -> p oc n", p=128)  # [128, 8, 4096]

    with tile.TileContext(nc) as tc:
        with (
            tc.tile_pool(name="consts", bufs=1) as consts,
            tc.tile_pool(name="x8p", bufs=NBLK) as x8p,
            tc.tile_pool(name="work", bufs=2) as work,
            tc.tile_pool(name="small", bufs=4) as small,
            tc.tile_pool(name="pbig", bufs=6, space="PSUM") as pbig,
        ):
            pkv = tc.alloc_tile_pool(name="pkv", bufs=1, space="PSUM")
            # ---- pass-A-critical loads, split across 4 DMA rings so the
            # first k-feature matmul's inputs land in ~2 ring-transfers
            # (per-ring bandwidth is ~110 GB/s; rings run in parallel).
            wtmp = consts.tile([128, 128], F16, name="wtmp")
            nc.vector.memset(wtmp[:], 0.125)
            eps_sb = consts.tile([128, 1], F32, name="eps_sb")
            nc.vector.memset(eps_sb[:], EPS)
            wkp8_sb = consts.tile([128, 8, 512], FP8, name="wkp8_sb")
            nc.scalar.dma_start(wkp8_sb[:, 0:4, :], wkp8[:, 0:4, :])
            nc.gpsimd.dma_start(wkp8_sb[:, 4:8, :], wkp8[:, 4:8, :])
            bkpb_sb = consts.tile([128, 8, 64], F16, name="bkpb_sb")
            nc.gpsimd.dma_start(bkpb_sb[:], bkpb)
            wv8_sb = consts.tile([128, 8, 512], FP8, name="wv8_sb")
            nc.scalar.dma_start(wv8_sb[:], wv8)
            ident_sb = consts.tile([128, 128], F16, name="ident_sb")
            nc.scalar.dma_start(ident_sb[:], ident)

            # pass-B constants ride the scalar ring AFTER the pass-A
            # critical set — ring FIFO keeps them from stealing bandwidth
            # from the transfers the first matmuls block on.
            wqp8_sb = consts.tile([128, 8, 512], FP8, name="wqp8_sb")
            nc.scalar.dma_start(wqp8_sb[:], wqp8)
            wp_sb = consts.tile([128, 4, 1024], F16, name="wp_sb")
            nc.scalar.dma_start(wp_sb[:], wp)
            bqpe_sb = consts.tile([128, 4], F32, name="bqpe_sb")
            nc.scalar.dma_start(bqpe_sb[:], bqpe)

            # kv accumulators: pairs (0,1) in kvacc0, (2,3) in kvacc1.
            # Layout per pair: 129 cols (64 v-head0 | 64 v-head1 | ksum), stride 130.
            kvacc = [
                pkv.tile([128, 260], F32, name=f"kvacc{t}", tag=f"kvacc{t}")
                for t in range(2)
            ]

            # PE warmup: keep the HAM activity window busy while the first
            # DMAs stream, so real matmuls start at 2.4 GHz.
            ps_warm = pbig.tile([128, 512], F32, name="ps_warm", tag="big")
            for _ in range(36):
                nc.tensor.matmul(
                    ps_warm[:, 0:128], wtmp[:], wtmp[:], start=True, stop=True
                )

            # ================= pass A: k', v -> kv, ksum =================
            def load_x8(blk):
                t = x8p.tile([128, 8, BLK], FP8, name="x8_t", tag="x8")
                nc.sync.dma_start(t[:], xt8[blk])
                return t

            def emit_v(x8_t, c, v_sbs):
                cs = slice(c * 128, (c + 1) * 128)
                psv = pbig.tile([128, 512], F32, name="ps_v", tag="big")
                for k in range(4):
                    nc.tensor.matmul(
                        psv[:],
                        x8_t[:, 2 * k : 2 * k + 2, cs],
                        wv8_sb[:, 2 * k : 2 * k + 2, :],
                        start=(k == 0),
                        stop=(k == 3),
                        perf_mode=DR,
                    )
                v_sb = work.tile([128, 4, 132], F16, name="v_sb", tag="v", bufs=5)
                nc.scalar.activation(
                    v_sb[:, :, 0:128],
                    psv.rearrange("p (g j) -> p g j", j=128),
                    AF.Copy,
                    scale=1.0 / WS,
                )
                nc.vector.memset(v_sb[:, :, 128:129], 1.0)
                v_sbs.append(v_sb)

            def emit_kf(x8_t, c, kp_sbs):
                cs = slice(c * 128, (c + 1) * 128)
                psf = pbig.tile([128, 512], F32, name="ps_kf", tag="big")
                for k in range(4):
                    nc.tensor.matmul(
                        psf[:],
                        x8_t[:, 2 * k : 2 * k + 2, cs],
                        wkp8_sb[:, 2 * k : 2 * k + 2, :],
                        start=(k == 0),
                        stop=(k == 3),
                        perf_mode=DR,
                    )
                psf_v = psf.rearrange("p (g f) -> p g f", f=64)  # [128, 8, 64]
                karg = small.tile([128, 8, 64], F16, name="karg", tag="karg")
                nc.vector.tensor_tensor(karg[:], psf_v, bkpb_sb[:], ALU.add)
                mx = small.tile([128, 8], F32, name="mx", tag="mx")
                nc.vector.reduce_max(mx[:], karg[:], axis=mybir.AxisListType.X)
                nc.gpsimd.tensor_tensor(
                    karg[:], karg[:],
                    mx[:, :, None].to_broadcast([128, 8, 64]),
                    ALU.subtract,
                )
                kp_sb = work.tile([128, 4, 128], F16, name="kp_sb", tag="kp", bufs=9)
                nc.scalar.activation(
                    kp_sb.rearrange("p g (h f) -> p (g h) f", f=64),
                    karg[:], AF.Exp, bias=eps_sb[:], scale=1.0 / WS,
                )
                kp_sbs.append(kp_sb)

            def emit_kv(blk, c, kp_sbs, v_sbs):
                glob_first = blk == 0 and c == 0
                glob_last = blk == NBLK - 1 and c == NCH - 1
                for p in range(4):
                    base = (p % 2) * 130
                    nc.tensor.matmul(
                        kvacc[p // 2][:, base : base + 129],
                        kp_sbs[c][:, p, :],
                        v_sbs[c][:, p, 0:129],
                        start=(glob_first and p % 2 == 0),
                        stop=(glob_last and p % 2 == 1),
                    )

            # blocks 0+1: k-features first (only need wkp8 + x8), so the PE
            # has work while wv8 is still streaming on the scalar ring.
            x8_blks = [load_x8(0), load_x8(1), load_x8(2), load_x8(3)]
            kp01 = [[], []]
            v01 = [[], []]
            for b in range(2):
                for c in range(NCH):
                    emit_kf(x8_blks[b], c, kp01[b])
            for b in range(2):
                for c in range(NCH):
                    emit_v(x8_blks[b], c, v01[b])
                for c in range(NCH):
                    emit_kv(b, c, kp01[b], v01[b])

            for blk in range(2, NBLK):
                if blk + 2 < NBLK:
                    x8_blks.append(load_x8(blk + 2))
                x8_t = x8_blks[blk]
                v_sbs, kp_sbs = [], []
                for c in range(NCH):
                    emit_kf(x8_t, c, kp_sbs)
                    emit_v(x8_t, c, v_sbs)
                for c in range(NCH):
                    emit_kv(blk, c, kp_sbs, v_sbs)

            # ======== boundary: ksum columns + wkvp = kv @ Wp fold ========
            # Pre-queue the first two blocks' q-feature GEMMs so the PE has
            # work while DVE/ACT assemble ksbc/wkvp.
            def emit_qp(blk):
                x8_t = x8_blks[blk]
                qp_sb = work.tile([128, 4, BLK], F16, name="qp_sb", tag="qp", bufs=6)
                for p in range(4):
                    ps = pbig.tile([128, BLK], F32, name="ps_qt", tag="big")
                    for k in range(4):
                        nc.tensor.matmul(
                            ps[:],
                            wqp8_sb[:, 2 * k : 2 * k + 2, p * 128 : (p + 1) * 128],
                            x8_t[:, 2 * k : 2 * k + 2, :],
                            start=(k == 0),
                            stop=(k == 3),
                            perf_mode=DR,
                        )
                    nc.scalar.activation(
                        qp_sb[:, p, :], ps[:], AF.Exp,
                        bias=bqpe_sb[:, p : p + 1], scale=1.0 / WS,
                    )
                return qp_sb

            qps = [emit_qp(0), emit_qp(1), emit_qp(2), emit_qp(3)]

            # ksbc_ext[hf, p, j]: ksum[hf] masked to head(j)'s block, the
            # stationary operand of the merged normalizer+broadcast matmul.
            ksbc = consts.tile([128, 4, 128], F16, name="ksbc")
            nc.vector.memset(ksbc[:], 0.0)
            for p in range(4):
                t = kvacc[p // 2]
                base = (p % 2) * 130
                ks = t[:, base + 128 : base + 129]
                nc.vector.tensor_copy(
                    out=ksbc[0:64, p, 0:64], in_=ks[0:64].to_broadcast([64, 64])
                )
                nc.vector.tensor_copy(
                    out=ksbc[64:128, p, 64:128], in_=ks[64:128].to_broadcast([64, 64])
                )

            # wkvp[hf, p, od] = sum_{vd in head(hf)} kv[hf,vd] Wp[vd,od].
            # (The v-bias needs no device-side term: softmax weights sum to
            # one, so bv contributes the constant bv@Wp, folded into b_proj
            # on the host.)  Stage-ordered so all kvacc reads finish first
            # and the per-pair chains pipeline across engines.
            wkvp_sb = consts.tile([128, 4, 1024], F16, name="wkvp_sb")
            kvsbs, psTs, kvTs = [], [], []
            for p in range(4):
                t = kvacc[p // 2]
                base = (p % 2) * 130
                # block-diagonal extract (off-diag quadrants are cross-head
                # garbage from the full outer-product accumulation)
                kvsb = small.tile([128, 128], F16, name="kvsb", tag="kvsb", bufs=4)
                nc.vector.memset(kvsb[:], 0.0)
                nc.vector.tensor_copy(out=kvsb[0:64, 0:64], in_=t[0:64, base : base + 64])
                nc.vector.tensor_copy(
                    out=kvsb[64:128, 64:128], in_=t[64:128, base + 64 : base + 128]
                )
                kvsbs.append(kvsb)
            for p in range(4):
                psT = pbig.tile([128, 128], F16, name="ps_T", tag="big")
                nc.tensor.transpose(psT[:], kvsbs[p][:], ident_sb[:])
                psTs.append(psT)
            for p in range(4):
                kvT = small.tile([128, 128], F16, name="kvT", tag="kvT", bufs=4)
                nc.scalar.copy(kvT[:], psTs[p][:])
                kvTs.append(kvT)
            for p in range(4):
                for half in range(2):
                    hs = slice(half * 512, (half + 1) * 512)
                    pw = pbig.tile([128, 512], F32, name="ps_w", tag="big")
                    nc.tensor.matmul(
                        pw[:], kvTs[p][:], wp_sb[:, p, hs], start=True, stop=True
                    )
                    if half == 0:
                        nc.vector.tensor_copy(out=wkvp_sb[:, p, hs], in_=pw[:])
                    else:
                        nc.scalar.copy(wkvp_sb[:, p, hs], pw[:])

            pkv.release()
            pnrm = tc.alloc_tile_pool(name="pnrm", bufs=2, space="PSUM")

            # ================= pass B: q' -> normalize -> proj =================
            def emit_pj(blk, q2s, oc_range):
                ns = slice(blk * BLK, (blk + 1) * BLK)
                for oc in oc_range:
                    pj = pbig.tile([128, BLK], F32, name="ps_pj", tag="big")
                    for p in range(4):
                        nc.tensor.matmul(
                            pj[:],
                            wkvp_sb[:, p, oc * 128 : (oc + 1) * 128],
                            q2s[p][:],
                            start=(p == 0),
                            stop=(p == 3),
                        )
                    pj_sb = small.tile([128, BLK], F16, name="pj_sb", tag="pj", bufs=6)
                    if oc % 2 == 0:
                        nc.vector.tensor_copy(out=pj_sb[:], in_=pj[:])
                    else:
                        nc.scalar.copy(pj_sb[:], pj[:])
                    eng = (nc.sync, nc.scalar, nc.gpsimd)[oc % 3]
                    eng.dma_start(out_v[:, oc, ns], pj_sb[:])

            def emit_nrm(blk):
                # merged normalizer+broadcast: one matmul per pair gives the
                # per-head norm already broadcast over its 64 partitions
                qp_sb = qps[blk]
                q2s = []
                for p in range(4):
                    nrm = pnrm.tile([128, BLK], F32, name="nrm", tag="nrm")
                    nc.tensor.matmul(
                        nrm[:], ksbc[:, p, :], qp_sb[:, p, :], start=True, stop=True
                    )
                    rec = small.tile([128, BLK], F32, name="rec", tag="rec", bufs=3)
                    nc.vector.reciprocal_approx_fast(out=rec[:], in_=nrm[:])
                    q2 = small.tile([128, BLK], F16, name="q2", tag="q2", bufs=9)
                    nc.vector.tensor_mul(q2[:], qp_sb[:, p, :], rec[:])
                    q2s.append(q2)
                return q2s

            prev_q2 = None  # (blk, q2s) whose proj is still pending
            for blk in range(NBLK - 1):
                q2s = emit_nrm(blk)
                if blk + 4 < NBLK:
                    qps.append(emit_qp(blk + 4))
                if prev_q2 is not None:
                    emit_pj(prev_q2[0], prev_q2[1], range(8))
                prev_q2 = (blk, q2s)

            # last block: interleave the two pending projs so the final
            # out-DMAs start ~4us earlier and the DMA rings drain in time
            q2s7 = emit_nrm(NBLK - 1)
            emit_pj(prev_q2[0], prev_q2[1], range(0, 4))
            emit_pj(NBLK - 1, q2s7, range(0, 2))
            emit_pj(prev_q2[0], prev_q2[1], range(4, 8))
            emit_pj(NBLK - 1, q2s7, range(2, 6))
            emit_pj(NBLK - 1, q2s7, range(6, 8))

            pnrm.release()

    nc.compile()
    return nc


_NC = None


def _get_nc():
    global _NC
    if _NC is None:
        _NC = _build_nc()
    return _NC


def _host_inputs(x, W_qkv, b_qkv, W_proj, b_proj, proj_mat):
    x = np.asarray(x, dtype=np.float32)
    W_qkv = np.asarray(W_qkv, dtype=np.float32)
    b_qkv = np.asarray(b_qkv, dtype=np.float32)
    W_proj = np.asarray(W_proj, dtype=np.float32)
    proj_mat = np.asarray(proj_mat, dtype=np.float32)

    pt = (proj_mat.T * SCALE).astype(np.float32)  # [hd, F]

    def tile_x(xb):
        # [NBLK, 128, 8, BLK]: contiguous per-partition runs for fast DMA
        xt = xb.T.reshape(8, 128, NBLK, BLK).transpose(2, 1, 0, 3)
        return np.ascontiguousarray(xt)

    def tile_w(w):
        # [D, 512] -> [128, 8, 512]
        return np.ascontiguousarray(w.reshape(8, 128, 512).transpose(1, 0, 2))

    xt8s = [tile_x(x[b]).astype(ml_dtypes.float8_e4m3) for b in range(4)]
    ident = np.eye(128, dtype=np.float16)

    def fuse(Wslc, bslc):
        # W_fused[:, (h f)] = sum_d W.T[:, (h d)] pt[d, f]; bias likewise
        wT = Wslc.T.reshape(D, 8, HD)
        wf = np.einsum("ahd,df->ahf", wT, pt).reshape(D, 512)
        bf = np.einsum("hd,df->hf", bslc.reshape(8, HD), pt).reshape(512)
        return wf, bf

    in_maps = []
    for c in range(8):
        b, g = c // 2, c % 2
        wqs = W_qkv[g * 512 : (g + 1) * 512]
        wks = W_qkv[D + g * 512 : D + (g + 1) * 512]
        wvs = W_qkv[2 * D + g * 512 : 2 * D + (g + 1) * 512]
        bqs = b_qkv[g * 512 : (g + 1) * 512]
        bks = b_qkv[D + g * 512 : D + (g + 1) * 512]
        bvs = b_qkv[2 * D + g * 512 : 2 * D + (g + 1) * 512]
        wqp, bqp = fuse(wqs, bqs)
        wkp, bkp = fuse(wks, bks)
        wp_loc = W_proj[:, g * 512 : (g + 1) * 512].T  # [512 vd, 1024 od]
        in_maps.append(
            {
                "xt8": xt8s[b],
                "wqp8": tile_w(wqp * WS).astype(ml_dtypes.float8_e4m3),
                "wkp8": tile_w(wkp * WS).astype(ml_dtypes.float8_e4m3),
                "wv8": tile_w(np.ascontiguousarray(wvs.T) * WS).astype(
                    ml_dtypes.float8_e4m3
                ),
                "wp": np.ascontiguousarray(
                    wp_loc.reshape(4, 128, 1024).transpose(1, 0, 2)
                ).astype(np.float16),
                "bqpe": np.ascontiguousarray(
                    (bqp + EPS).reshape(4, 128).T
                ).astype(np.float32),
                "bkpb": np.ascontiguousarray(
                    np.broadcast_to(bkp.reshape(1, 8, 64) * WS, (128, 8, 64))
                ).astype(np.float16),
                "ident": ident,
            }
        )
    return in_maps


def kernel(x, W_qkv, b_qkv, W_proj, b_proj, proj_mat):
    b_proj = np.asarray(b_proj, dtype=np.float32)
    b_qkv = np.asarray(b_qkv, dtype=np.float32)
    W_proj = np.asarray(W_proj, dtype=np.float32)
    in_maps = _host_inputs(x, W_qkv, b_qkv, W_proj, b_proj, proj_mat)
    nc = _get_nc()
    res = run_bass_kernel_spmd(nc, in_maps, core_ids=list(range(8)))
    # softmax weights sum to 1, so the v-bias passes through attention
    # unchanged and lands as the constant bv @ Wp^T
    b_eff = b_proj + b_qkv[2 * D :] @ W_proj.T
    final = np.empty((4, N, D), dtype=np.float32)
    for b in range(4):
        acc = res.results[2 * b]["out"].astype(np.float32) + res.results[
            2 * b + 1
        ]["out"].astype(np.float32)
        final[b] = acc.T + b_eff[None, :]
    return final
